# revision 1
# baseline (speedup 1.0000x reference)
"""Trainium2 Bass kernel for GNN message passing (APPR-style aggregation).

Computes: out = x + 0.15 * segment_sum(x[src], dst, num_segments=N)
for x [100000, 64] f32 and edge_index [2, 1600000] int64.

Strategy (8 NeuronCores, no collectives needed):
  - Host shards EDGES by destination-owner core (core c owns nodes
    [c*12500, (c+1)*12500)); within a core, edges are bucketed by
    128-node destination block and by source quadrant (x split into 4
    row-quadrants so dma_gather's int16 indices can address it).
  - On device: per (block-GROUP, quadrant) mega dma_gather of 0.15*x[src]
    rows (bf16, rows padded to 256 B) into SBUF. Each dma_gather pays a
    ~0.5-1 us fixed cost on GPSIMD, so few big gathers (~52/core, ~4.3k
    rows each) cut the GPSIMD wall from ~700 us (previous ~600 small
    gathers) to under 100 us of true emission time.
  - Slot layout is run-aligned, not tile-aligned: each (block, quad) run
    has length max-over-cores(count) with interior pads (index 0,
    dst-code 255); tiles may straddle two blocks. Each block's one-hot
    selection uses its own dcol columns (straddle tiles get a duplicated
    column per block with the other block's rows masked to 255), so one
    DVE is_equal per block builds the whole selection matrix, and one
    matmul per (block, spanned tile) accumulates S^T @ gathered into an
    f32 PSUM tile. Epilogue adds the f32 x slice and DMAs the block out.
  - All 8 cores run the same static graph (counts maxed over cores);
    per-core pads gather row 0 and carry code 255 so they contribute 0.

  Perf notes (measured on HW, 523 us):
  - The pipeline is paced by the gather DMA TRANSFER drain (~215k rows x
    256 B per core; 256 B descriptors run at ~1/2 bus rate => ~180 GB/s
    aggregate, ~45 GB/s per SWDGE queue) co-equal with DVE sel
    construction (is_equal runs 1x: broadcast operands disqualify the
    2x packing mode; ~3.4 us per block). Gather-instruction durations
    show [~35us, 1, 1, 1] per group: the first gather of a group blocks
    on the gather-pool WAR (consumers of group g-3), not on emission.
  - Measured dead ends: ap_gather 27.8 ns/row, indirect_dma_start 19.4
    ns/row (both >> dma_gather), single_packet=True crashes the device,
    trailing -1 idx skips are slower AND risk 0*NaN from uninitialized
    SBUF, and folding the x-add into the PSUM chain via identity matmul
    regresses (pulls the x DMA into every block's critical PE chain).
  - gather pool bufs=3 (not 2) removed ~50 us of inter-gather stalls.
"""

import math
import os
import sys
import types

import numpy as np

for _p in ("/opt/trn_rl_repo", "/root/.axon_site/_ro/trn_rl_repo"):
    if os.path.isdir(_p) and _p not in sys.path:
        sys.path.append(_p)

import ml_dtypes
import concourse.bass as bass
import concourse.mybir as mybir
import concourse.tile as tile
from concourse import bacc
from concourse.bass_utils import run_bass_kernel_spmd
from concourse.vector_clock import ScopedClock

WEIGHT = 0.15
N_NODES = 100000
D_FEAT = 64
N_CORES = 8
P = 128
NQUAD = 4
ROWPAD = 128  # gathered bf16 row padded to 128 elems = 256 B
NPC = N_NODES // N_CORES  # nodes per core
NBLK = (NPC + P - 1) // P  # 128-node dst blocks per core
NQROWS = N_NODES // NQUAD  # rows per source quadrant (must fit int16)

GRP = int(os.environ.get("BASS_GRP", "8"))  # dst blocks per gather group
# max tiles per dma_gather instruction (split cap; 512 => effectively off)
CHUNK_TILES = int(os.environ.get("BASS_CHUNK_TILES", "512"))
NMETA = 8  # srci is loaded in this many chunks so gathers start early
SINGLE_PACKET = os.environ.get("BASS_SINGLE_PACKET", "0") == "1"

LAST_EXEC_TIME_NS = None

MAX_WAITS = 2  # this walrus build rejects instructions with more sync commands


def _patch_tile_drain():
    """This walrus build rejects >MAX_WAITS sync commands (waits+updates)
    on one instruction. Two patches: (a) the tail drain re-emits its waits
    as individual wait_ge instructions; (b) any scheduled instruction with
    too many waits gets the excess hoisted onto same-engine InstNoOps
    placed immediately before it."""
    if getattr(tile.TileContext, "_drain_patched", False):
        return

    def _drain_and_barrier(self, tick_clock, wait_clock):
        drain_inst = self.nc.sync.drain()
        wait_clock.add_sem_waits(
            drain_inst.ins, ScopedClock({None: tick_clock.global_clock})
        )
        si = drain_inst.ins.sync_info
        waits = list(si.on_wait) if si is not None else []
        if len(waits) > MAX_WAITS:
            drain_inst.ins.sync_info = mybir.SyncInfo(on_wait=[], on_update=[])
            handles = {h.name: h for h in wait_clock.sems.allocated().values()}
            for w in waits:
                self.nc.sync.wait_ge(handles[w.ant_name], w.wait_value)
            self.nc.sync.drain()
        self.nc.all_engine_barrier()
        popped = self.nc._tile_sem_poison_stack.pop()
        assert popped is self._sem_poison
        self.nc.clear_and_free_semaphores(list(self.sems.allocated().values()))
        self.nc.all_engine_barrier()

    orig_lower = tile.TileContext._lower_ordered_insts

    def _lower_ordered_insts(self, ordered):
        for bb_name, insts in ordered.items():
            new_list = []
            for inst in insts:
                si = getattr(inst, "sync_info", None)
                n_w = len(si.on_wait) if si is not None and si.on_wait else 0
                n_u = len(si.on_update) if si is not None and si.on_update else 0
                budget = max(0, MAX_WAITS - n_u)
                if (
                    n_w > budget
                    and type(inst).__name__.startswith("Inst")
                    and inst.engine is not None
                ):
                    waits = list(si.on_wait)
                    keep = waits[len(waits) - budget :] if budget else []
                    excess = waits[: len(waits) - budget]
                    for w in excess:
                        nop = mybir.InstNoOp(
                            name=self.nc.get_next_instruction_name(),
                            sync_info=mybir.SyncInfo(on_wait=[w], on_update=[]),
                            engine=inst.engine,
                            bass_nofuse=True,
                        )
                        new_list.append(nop)
                    inst.sync_info = mybir.SyncInfo(
                        on_wait=keep, on_update=list(si.on_update)
                    )
                new_list.append(inst)
            insts[:] = new_list
        return orig_lower(self, ordered)

    tile.TileContext._drain_and_barrier = _drain_and_barrier
    tile.TileContext._lower_ordered_insts = _lower_ordered_insts
    tile.TileContext._drain_patched = True


def _install_ntff_hook():
    """Register the NTFF profiling hook that this container's boot skips
    (antenv.axon_hooks missing). Only needed when tracing is requested."""
    if "antenv.axon_hooks" in sys.modules:
        return
    try:
        from trn_agent_boot.trn_boot import _ntff_profile_via_ctypes

        hook = _ntff_profile_via_ctypes("/opt/axon/libaxon_pjrt.so")
        if hook is None:
            return
        mod = types.ModuleType("antenv.axon_hooks")
        mod._hook = hook
        mod.get_axon_ntff_profile_hook = lambda: mod._hook
        mod.set_axon_ntff_profile_hook = lambda h: setattr(mod, "_hook", h)
        sys.modules["antenv.axon_hooks"] = mod
        import antenv

        antenv.axon_hooks = mod
    except Exception as e:  # profiling is optional
        print(f"ntff hook install failed: {e}", file=sys.stderr)


class Plan:
    """Static (core-independent) layout derived from max-over-core counts."""

    def __init__(self, maxc):
        maxc = maxc.copy()
        for b in range(NBLK):  # every block needs >= 1 tile for its PSUM chain
            if maxc[b].sum() == 0:
                maxc[b, 0] = 1
        self.maxc = maxc  # [NBLK, NQUAD]

        # group sizes: GRP-block groups, but taper the last two groups so the
        # post-last-gather consumption tail is short
        sizes = []
        left = NBLK
        while left > 0:
            if left > GRP:
                sizes.append(GRP)
                left -= GRP
            elif left > GRP // 2:
                sizes.append((left + 1) // 2 + 1)
                left -= sizes[-1]
            else:
                sizes.append(left)
                left = 0
        self.group_sizes = sizes
        self.group_first = np.concatenate([[0], np.cumsum(sizes)])[:-1]
        self.group_of = np.zeros(NBLK, dtype=np.int64)
        for g, (f, s) in enumerate(zip(self.group_first, sizes)):
            self.group_of[f : f + s] = g
        self.ngroups = len(sizes)
        ng, nq = self.ngroups, NQUAD

        # rows and tiles per (group, quad) region
        self.R = np.zeros((ng, nq), dtype=np.int64)
        for g in range(ng):
            f, s = int(self.group_first[g]), sizes[g]
            self.R[g] = maxc[f : f + s].sum(axis=0)
        self.T = (self.R + P - 1) // P
        # gather order: g-major, q-minor
        self.tile_base = np.zeros((ng, nq), dtype=np.int64)
        self.tile_base.ravel()[1:] = np.cumsum(self.T.ravel())[:-1]
        self.t_total = int(self.T.sum())
        self.group_tile0 = self.tile_base[:, 0]  # first tile of group g
        self.group_tiles = self.T.sum(axis=1)  # tiles per group

        # run offset of (b, q) within its region
        self.run_off = np.zeros_like(maxc)
        for g in range(ng):
            sl = slice(int(self.group_first[g]), int(self.group_first[g]) + sizes[g])
            c = np.cumsum(maxc[sl], axis=0)
            self.run_off[sl][1:] = c[:-1]

        # per-block spanned tiles (global tile ids) and dcol column layout
        self.block_tiles = []  # list over b of list of global tile ids
        self.dci_base = np.zeros((NBLK, nq), dtype=np.int64)
        self.first_tile = np.zeros((NBLK, nq), dtype=np.int64)
        ncol = 0
        for b in range(NBLK):
            g = int(self.group_of[b])
            tl = []
            for q in range(nq):
                if maxc[b, q] == 0:
                    self.dci_base[b, q] = ncol
                    self.first_tile[b, q] = -1
                    continue
                ft = self.tile_base[g, q] + self.run_off[b, q] // P
                lt = self.tile_base[g, q] + (self.run_off[b, q] + maxc[b, q] - 1) // P
                self.dci_base[b, q] = ncol
                self.first_tile[b, q] = ft
                ncol += lt - ft + 1
                tl.extend(range(int(ft), int(lt) + 1))
            self.block_tiles.append(tl)
        self.ncol = ncol
        self.span_max = max(len(tl) for tl in self.block_tiles)
        self.t_gmax = int(self.group_tiles.max())

        # gather chunks: (g, q, tile_offset_in_region, n_tiles). Every slot
        # is emitted (pads gather row 0): trailing -1 skips measured SLOWER
        # on HW and risk 0*NaN poisoning from uninitialized SBUF.
        self.FULL_GROUPS = ng
        self.chunks = []
        vc = []
        for g in range(ng):
            for q in range(nq):
                tn = int(self.T[g, q])
                done = 0
                while done < tn:
                    ct = min(CHUNK_TILES, tn - done)
                    self.chunks.append((g, q, done, ct))
                    vc.append(ct * P)
                    done += ct
        self.vc = np.array([vc], dtype=np.int32)

        # srci load chunks: group ranges split into NMETA pieces
        self.meta_ranges = []  # (first_tile, n_tiles) per piece
        gsplit = np.array_split(np.arange(ng), min(NMETA, ng))
        self.meta_of_group = np.zeros(ng, dtype=np.int64)
        for mi, gs in enumerate(gsplit):
            t0 = int(self.tile_base[gs[0], 0])
            tn = int(self.group_tiles[gs].sum())
            self.meta_ranges.append((t0, tn))
            self.meta_of_group[gs] = mi


def _preprocess(edge_index):
    """Bucket edges per (core, dst-block, src-quadrant); build device
    input arrays in the run-aligned slot order the device graph consumes."""
    src = np.asarray(edge_index[0]).astype(np.int64)
    dst = np.asarray(edge_index[1]).astype(np.int64)
    E = src.shape[0]

    core = dst // NPC
    local = dst - core * NPC
    blk = local >> 7
    col = local & 127
    quad = src // NQROWS
    loc = (src - quad * NQROWS).astype(np.int64)

    gkey = (core * NBLK + blk) * NQUAD + quad
    order = np.argsort(gkey, kind="stable")
    gkey_s = gkey[order]
    loc_s = loc[order]
    col_s = col[order]

    counts = np.bincount(gkey, minlength=N_CORES * NBLK * NQUAD).reshape(
        N_CORES, NBLK, NQUAD
    )
    maxc = counts.max(axis=0)  # [NBLK, NQUAD]
    plan = Plan(maxc)

    group_starts = np.zeros(N_CORES * NBLK * NQUAD + 1, dtype=np.int64)
    np.cumsum(counts.ravel(), out=group_starts[1:])
    j = np.arange(E) - group_starts[gkey_s]  # rank within (core, b, q)
    bq_s = gkey_s % (NBLK * NQUAD)
    core_s = gkey_s // (NBLK * NQUAD)
    b_s = bq_s // NQUAD
    q_s = bq_s % NQUAD
    g_s = plan.group_of[b_s]

    slot = (
        plan.tile_base[g_s, q_s] * P + plan.run_off[b_s, q_s] + j
    )  # global slot id

    # int16 gather indices: slot i -> partition i%16, column i//16
    idx16 = np.zeros((N_CORES, 16, plan.t_total * 8), dtype=np.int16)
    idx16[core_s, slot & 15, slot >> 4] = loc_s
    idx_arr = np.tile(idx16, (1, 8, 1))

    # dcol: per-block span columns; pads/other-block rows stay 255
    dcol = np.full((N_CORES, P, plan.ncol), 255.0, dtype=ml_dtypes.bfloat16)
    dci = plan.dci_base[b_s, q_s] + (slot >> 7) - plan.first_tile[b_s, q_s]
    dcol[core_s, slot & 127, dci] = col_s.astype(ml_dtypes.bfloat16)

    return idx_arr, dcol, plan


def _build_graph(plan):
    nc = bacc.Bacc(num_swdge_queues=4, dynamic_dma_scratch_size=32768)
    f32 = mybir.dt.float32
    bf16 = mybir.dt.bfloat16
    xq_p = [
        nc.declare_dram_parameter(f"xq{q}", [NQROWS, ROWPAD], bf16, isOutput=False)
        for q in range(NQUAD)
    ]
    xsl_p = nc.declare_dram_parameter("xsl", [NPC, D_FEAT], f32, isOutput=False)
    srci_p = nc.declare_dram_parameter(
        "srci", [P, plan.t_total * 8], mybir.dt.int16, isOutput=False
    )
    n_chunks = len(plan.chunks)
    vc_p = nc.declare_dram_parameter("vc", [1, n_chunks], mybir.dt.int32, isOutput=False)
    dcol_p = nc.declare_dram_parameter("dcol", [P, plan.ncol], bf16, isOutput=False)
    iota_p = nc.declare_dram_parameter("iota", [P, P], bf16, isOutput=False)
    out_p = nc.declare_dram_parameter("out", [NPC, D_FEAT], f32, isOutput=True)

    # chunks grouped by g for the build loop (chunks are (g,q)-ordered)
    chunks_by_group = [[] for _ in range(plan.ngroups)]
    for gi, (g, q, done, ct) in enumerate(plan.chunks):
        chunks_by_group[g].append((gi, q, done, ct))

    with tile.TileContext(nc) as tc:
        with (
            nc.gpsimd.register("vreg0") as vreg0,
            nc.gpsimd.register("vreg1") as vreg1,
            nc.gpsimd.register("vreg2") as vreg2,
            nc.gpsimd.register("vreg3") as vreg3,
            tc.tile_pool(name="const", bufs=1) as const_tp,
            tc.tile_pool(name="meta", bufs=1) as meta_tp,
            tc.tile_pool(name="gather", bufs=3) as gather_tp,
            tc.tile_pool(name="sel", bufs=4) as sel_tp,
            tc.tile_pool(name="xin", bufs=4) as xin_tp,
            tc.tile_pool(name="osb", bufs=4) as osb_tp,
            tc.tile_pool(name="psum", bufs=8, space="PSUM") as psum_tp,
        ):
            # load order matters: the first gather needs only vc + idx chunk 0,
            # so issue those first and the bulky dcol/late srci chunks after
            vc_sb = meta_tp.tile([1, n_chunks], mybir.dt.int32)
            nc.sync.dma_start(out=vc_sb[:], in_=vc_p[:])
            idx_tiles = []
            for mi, (t0, tn) in enumerate(plan.meta_ranges):
                it = meta_tp.tile([P, tn * 8], mybir.dt.int16, tag=f"idx{mi}")
                idx_tiles.append(it)
            nc.sync.dma_start(
                out=idx_tiles[0][:],
                in_=srci_p[:, plan.meta_ranges[0][0] * 8 :
                           (plan.meta_ranges[0][0] + plan.meta_ranges[0][1]) * 8],
            )
            iota_sb = const_tp.tile([P, P], bf16)
            nc.sync.dma_start(out=iota_sb[:], in_=iota_p[:])
            dcol_sb = meta_tp.tile([P, plan.ncol], bf16)
            nc.sync.dma_start(out=dcol_sb[:], in_=dcol_p[:])
            for mi, (t0, tn) in enumerate(plan.meta_ranges[1:], start=1):
                nc.sync.dma_start(
                    out=idx_tiles[mi][:], in_=srci_p[:, t0 * 8 : (t0 + tn) * 8]
                )

            vregs = [vreg0, vreg1, vreg2, vreg3]
            gi_global = 0
            for g in range(plan.ngroups):
                gt0 = int(plan.group_tile0[g])
                mi = int(plan.meta_of_group[g])
                m_t0, _ = plan.meta_ranges[mi]
                idx_sb = idx_tiles[mi]

                Gt = gather_tp.tile([P, plan.t_gmax * ROWPAD], bf16, tag="g")
                for ci, (gi, q, done, ct) in enumerate(chunks_by_group[g]):
                    off = int(plan.tile_base[g, q]) - gt0 + done
                    o16 = (int(plan.tile_base[g, q]) - m_t0 + done) * 8
                    if gi_global % 4 == 0:
                        nb = min(4, n_chunks - gi)
                        nc.gpsimd.load(vregs[:nb], vc_sb[0:1, gi : gi + nb])
                    nc.gpsimd.dma_gather(
                        out_ap=Gt[
                            :, off * ROWPAD : (off + ct) * ROWPAD
                        ].rearrange("p (c d) -> p c d", d=ROWPAD),
                        in_ap=xq_p[q][:, :],
                        idxs_ap=idx_sb[:, o16 : o16 + ct * 8],
                        num_idxs=ct * P,
                        num_idxs_reg=vregs[gi_global % 4],
                        elem_size=ROWPAD,
                        queue_num=gi_global % 4,
                        single_packet=SINGLE_PACKET,
                    )
                    gi_global += 1

                for b in range(
                    int(plan.group_first[g]),
                    int(plan.group_first[g]) + plan.group_sizes[g],
                ):
                    tl = plan.block_tiles[b]
                    sb = len(tl)
                    dci0 = int(plan.dci_base[b, 0])
                    rows = min(P, NPC - b * P)

                    sel = sel_tp.tile([P, plan.span_max * P], bf16, tag="s")
                    nc.vector.tensor_tensor(
                        out=sel[:, : sb * P].rearrange("p (t n) -> p t n", n=P),
                        in0=dcol_sb[:, dci0 : dci0 + sb]
                        .unsqueeze(2)
                        .to_broadcast([P, sb, P]),
                        in1=iota_sb[:].unsqueeze(1).to_broadcast([P, sb, P]),
                        op=mybir.AluOpType.is_equal,
                    )

                    ps = psum_tp.tile([P, D_FEAT], f32, space="PSUM", tag="ps")
                    for k, tglob in enumerate(tl):
                        toff = tglob - gt0
                        nc.tensor.matmul(
                            out=ps[:],
                            lhsT=sel[:, k * P : (k + 1) * P],
                            rhs=Gt[:, toff * ROWPAD : toff * ROWPAD + D_FEAT],
                            start=(k == 0),
                            stop=(k == len(tl) - 1),
                        )

                    xt = xin_tp.tile([P, D_FEAT], f32, tag="x")
                    nc.sync.dma_start(
                        out=xt[:rows], in_=xsl_p[b * P : b * P + rows, :]
                    )
                    ot = osb_tp.tile([P, D_FEAT], f32, tag="o")
                    nc.vector.tensor_add(out=ot[:rows], in0=xt[:rows], in1=ps[:rows])
                    nc.sync.dma_start(
                        out=out_p[b * P : b * P + rows, :], in_=ot[:rows]
                    )
    nc.compile()
    return nc


def kernel(x, edge_index):
    global LAST_EXEC_TIME_NS
    _patch_tile_drain()

    x = np.ascontiguousarray(np.asarray(x, dtype=np.float32))
    idx_arr, dcol_arr, plan = _preprocess(edge_index)

    xq = np.zeros((N_NODES, ROWPAD), dtype=ml_dtypes.bfloat16)
    xq[:, :D_FEAT] = (x * np.float32(WEIGHT)).astype(ml_dtypes.bfloat16)
    iota = np.broadcast_to(
        np.arange(P, dtype=np.float32).astype(ml_dtypes.bfloat16), (P, P)
    ).copy()

    nc = _build_graph(plan)

    in_maps = []
    for c in range(N_CORES):
        m = {
            "xsl": np.ascontiguousarray(x[c * NPC : (c + 1) * NPC]),
            "srci": np.ascontiguousarray(idx_arr[c]),
            "dcol": np.ascontiguousarray(dcol_arr[c]),
            "vc": np.ascontiguousarray(plan.vc),
            "iota": iota,
        }
        for q in range(NQUAD):
            m[f"xq{q}"] = np.ascontiguousarray(xq[q * NQROWS : (q + 1) * NQROWS])
        in_maps.append(m)

    trace = bool(os.environ.get("BASS_KERNEL_TRACE"))
    if trace:
        _install_ntff_hook()
    res = run_bass_kernel_spmd(
        nc, in_maps, core_ids=list(range(N_CORES)), trace=trace
    )
    LAST_EXEC_TIME_NS = res.exec_time_ns

    out = np.concatenate([res.results[c]["out"] for c in range(N_CORES)], axis=0)
    return out.astype(np.float32)

